# revision 1
# baseline (speedup 1.0000x reference)
"""Cached self-attention (QK-RMSNorm + RoPE + extend-cache MHA + out-proj),
tensor-parallel over heads across 8 trn2 NeuronCores.

Sharding: Wq/Wk/Wv column-sharded (3 heads = 384 dims per core), Wo
row-sharded; each core owns its slice of the KV cache. The QK RMSNorm is over
the full 3072-dim vector, so per-core partial sum-of-squares are AllReduced
(tiny [128,8] tensor). The output projection produces per-core partial sums
over the full model dim which the host reduces (the "all-reduce after the
output projection" done host-side, where it is free).

Precision: projections / out-proj run in float32r (TensorE tf32-like mode,
full rate at moving-dim >= 256); the attention streams (K/V cache, new q/k
heads, exp(probs)) are bf16, which halves the dominant HBM traffic; all
accumulation is fp32 in PSUM. Measured end-to-end relative error ~3e-3 of
absmax vs the fp32 jax reference (resid_var ~1e-7, far under the 1e-4 norm).

Device layouts (host pre-arranges everything so every DMA is contiguous):
  xT   [128, 24, 512]    x.T partition-tiled; rows r = b*256 + s
  wqT/wkT/wvT [128, 24, 384]  W[c_slice, :].T partition-tiled (f32r)
  woT  [128, 3, 3072]    Wo[:, c_slice].T partition-tiled (f32r)
  kTc  [2, 3, 128, 8192] cached K head-transposed, bf16 (hd on partitions)
  vc   [2, 3, 2, 128, 32, 128] cached V pre-tiled to SBUF layout, bf16
Attention per (b, h): scoresT[s, q] = k @ qT (PE), two s-tiles' scores share
one PSUM bank so one ACT exp call covers a [128, 512] pair (amortizes the
~350-cycle ACT fixed cost); p = exp(scale*scoresT) in bf16 (no
max-subtraction needed: |scores|*scale ~ N(0,1), exp is safe in fp32);
out[hd, q] += vT@p (PE accumulate); denom[q] += ones.T@p (PE M=1, one N=512
matmul per pair); normalize by 1/denom (partition-broadcast + DVE). The PE
stream is software-pipelined: pair t+1's score matmuls are emitted before
pair t's V/denom matmuls so ACT latency never stalls the PE.
"""

import ml_dtypes
import numpy as np

import concourse.bass as bass
import concourse.mybir as mybir
import concourse.tile as tile
from concourse import bacc
from concourse.bass import ts
from concourse.bass_utils import run_bass_kernel_spmd
from concourse.masks import make_identity

F32 = mybir.dt.float32
F32R = mybir.dt.float32r
BF16 = mybir.dt.bfloat16
AF = mybir.ActivationFunctionType
OP = mybir.AluOpType

B = 2
S_NEW = 256
DIM = 3072
NUM_HEADS = 24
HD = 128
EPS = 1e-6
NCORES = 8
HL = NUM_HEADS // NCORES  # heads per core: 3
CD = HL * HD  # per-core head dims: 384
R = B * S_NEW  # 512 query rows, r = b*256 + s
RC = R // 128  # 4 row chunks
NI = DIM // 128  # 24 contraction chunks
SCALE = 1.0 / np.sqrt(HD)


def build(s_cached: int, s_chunk: int, collective: bool = True):
    """Build the per-core SPMD module. s_cached/s_chunk parameterized so a
    scaled-down variant can run under CoreSim."""
    n_sc = s_cached // s_chunk
    tpc = s_chunk // 128  # s-tiles per chunk
    assert s_cached % s_chunk == 0 and s_chunk % 256 == 0, (
        "pairing assumes an even number of 128-row s-tiles per chunk"
    )
    nc = bacc.Bacc("TRN2", target_bir_lowering=False, debug=False, num_devices=NCORES)

    xT = nc.declare_dram_parameter("xT", [128, NI, R], F32R, isOutput=False)
    wqT = nc.declare_dram_parameter("wqT", [128, NI, CD], F32R, isOutput=False)
    wkT = nc.declare_dram_parameter("wkT", [128, NI, CD], F32R, isOutput=False)
    wvT = nc.declare_dram_parameter("wvT", [128, NI, CD], F32R, isOutput=False)
    woT = nc.declare_dram_parameter("woT", [128, HL, DIM], F32R, isOutput=False)
    kTc = nc.declare_dram_parameter("kTc", [B, HL, HD, s_cached], BF16, isOutput=False)
    vc = nc.declare_dram_parameter(
        "vc", [B, HL, s_cached // s_chunk, 128, s_chunk // 128, 128], BF16, isOutput=False
    )
    cosb = nc.declare_dram_parameter("cosb", [128, RC, CD // 2], F32, isOutput=False)
    sinb = nc.declare_dram_parameter("sinb", [128, RC, CD // 2], F32, isOutput=False)
    gq = nc.declare_dram_parameter("gq", [1, CD], F32, isOutput=False)
    gk = nc.declare_dram_parameter("gk", [1, CD], F32, isOutput=False)
    bq = nc.declare_dram_parameter("bq", [1, CD], F32, isOutput=False)
    bk = nc.declare_dram_parameter("bk", [1, CD], F32, isOutput=False)
    bv = nc.declare_dram_parameter("bv", [1, CD], F32, isOutput=False)
    ones_in = nc.declare_dram_parameter("ones_in", [128, 1], BF16, isOutput=False)
    out_d = nc.declare_dram_parameter("out", [R, DIM], F32, isOutput=True)

    with tile.TileContext(nc) as tc:
        with (
            tc.tile_pool(name="const", bufs=1) as const,
            tc.tile_pool(name="dram", bufs=1, space="DRAM") as dram,
            tc.tile_pool(name="qkT", bufs=1) as pqkT,
            tc.tile_pool(name="vsb", bufs=1) as pvs,
            tc.tile_pool(name="attn", bufs=1) as pattn,
            ):
            # ---- constants ----
            ident = const.tile([128, 128], F32)
            make_identity(nc, ident)
            eps_t = const.tile([128, 1], F32)
            nc.vector.memset(eps_t, EPS)
            ones_t = const.tile([128, 1], BF16)
            nc.sync.dma_start(out=ones_t, in_=ones_in[:])
            cos_t = const.tile([128, RC, CD // 2], F32)
            sin_t = const.tile([128, RC, CD // 2], F32)
            nc.sync.dma_start(out=cos_t, in_=cosb[:])
            nc.sync.dma_start(out=sin_t, in_=sinb[:])
            bcasts = {}
            for name, src in (("gq", gq), ("gk", gk), ("bq", bq), ("bk", bk), ("bv", bv)):
                t = const.tile([128, CD], F32, tag=f"bc_{name}")
                nc.gpsimd.dma_start(out=t, in_=src[:].to_broadcast((128, CD)))
                bcasts[name] = t

            # persistent activations
            q_kT = pqkT.tile([128, 2 * HL, R], BF16)  # [hd, 0:3 qheads | 3:6 kheads, r]
            vs = pvs.tile([128, RC, CD], BF16)  # new V natural
            attn_sb = pattn.tile([128, B * HL, S_NEW], F32R)  # normalized attn outT

            with (
                tc.tile_pool(name="xt", bufs=1) as px,
                tc.tile_pool(name="wstream", bufs=2) as pw,
                tc.tile_pool(name="projps", bufs=4, space="PSUM") as pp,
                tc.tile_pool(name="qknat", bufs=1) as pqk,
                tc.tile_pool(name="scratch", bufs=2) as scratch,
                tc.tile_pool(name="stats", bufs=1) as pstats,
                tc.tile_pool(name="tps", bufs=2, space="PSUM") as ptp,
            ):
                xt = px.tile([128, NI, R], F32R)
                for xc in range(3):
                    nc.sync.dma_start(
                        out=xt[:, ts(xc, NI // 3), :], in_=xT[:, ts(xc, NI // 3), :]
                    )

                qs = pqk.tile([128, RC, CD], F32, tag="qs")
                ks = pqk.tile([128, RC, CD], F32, tag="ks")
                ssq = pstats.tile([128, 8], F32, tag="ssq")
                ssq_red = pstats.tile([128, 8], F32, tag="ssq_red")
                rstd = pstats.tile([128, 8], F32, tag="rstd")

                def projection(wT_d, nat_out, bias_t, ssq_col):
                    wr = wT_d[:]
                    psums = [
                        pp.tile([128, CD], F32, name="projps", tag="projps")
                        for rc in range(RC)
                    ]
                    for ic in range(NI // 8):
                        w_t = pw.tile([128, 8, CD], F32R)
                        nc.sync.dma_start(out=w_t, in_=wr[:, ts(ic, 8), :])
                        for ii in range(8):
                            i = ic * 8 + ii
                            for rc in range(RC):
                                nc.tensor.matmul(
                                    out=psums[rc],
                                    lhsT=xt[:, i, ts(rc, 128)],
                                    rhs=w_t[:, ii, :],
                                    start=(i == 0),
                                    stop=(i == NI - 1),
                                )
                    for rc in range(RC):
                        nc.vector.tensor_add(
                            out=nat_out[:, rc, :], in0=psums[rc], in1=bias_t
                        )
                        if ssq_col is not None:
                            # (tensor_tensor_reduce wedges the device; use
                            # square + reduce_sum instead)
                            sq = scratch.tile([128, CD], F32, tag="sq")
                            nc.vector.tensor_mul(
                                out=sq, in0=nat_out[:, rc, :], in1=nat_out[:, rc, :]
                            )
                            nc.vector.reduce_sum(
                                out=ssq[:, ssq_col + rc : ssq_col + rc + 1],
                                in_=sq[:],
                                axis=mybir.AxisListType.X,
                            )

                projection(wqT, qs, bcasts["bq"], 0)
                projection(wkT, ks, bcasts["bk"], 4)

                # tiny AllReduce of the norm statistics
                cc_in = dram.tile([128, 8], F32)
                cc_out = dram.tile([128, 8], F32)
                nc.sync.dma_start(out=cc_in[:], in_=ssq)
                if collective:
                    nc.gpsimd.collective_compute(
                        "AllReduce",
                        OP.add,
                        replica_groups=[list(range(NCORES))],
                        ins=[cc_in.opt()],
                        outs=[cc_out.opt()],
                    )
                else:
                    nc.sync.dma_start(out=cc_out[:], in_=cc_in[:])
                nc.sync.dma_start(out=ssq_red, in_=cc_out[:])

                # V projection (no dependency on the AllReduce; fills the wait)
                projection(wvT, vs, bcasts["bv"], None)

                # rstd = 1/sqrt(ssq/DIM + eps)
                nc.scalar.activation(
                    out=rstd, in_=ssq_red, func=AF.Sqrt, bias=eps_t, scale=1.0 / DIM
                )
                nc.vector.reciprocal(out=rstd, in_=rstd)

                # norm + rope on q, k
                for nat, rop, gname, col0 in (
                    (qs, qs, "gq", 0),
                    (ks, ks, "gk", 4),
                ):
                    for rc in range(RC):
                        nrm = scratch.tile([128, CD], F32, tag="nrm")
                        nc.vector.tensor_scalar_mul(
                            out=nrm,
                            in0=nat[:, rc, :],
                            scalar1=rstd[:, col0 + rc : col0 + rc + 1],
                        )
                        gsc = scratch.tile([128, CD], F32, tag="gsc")
                        nc.vector.tensor_mul(out=gsc, in0=nrm, in1=bcasts[gname])
                        gp = gsc.rearrange("p (j two) -> p j two", two=2)
                        rp = rop[:, rc, :].rearrange("p (j two) -> p j two", two=2)
                        ce = cos_t[:, rc, :]
                        se = sin_t[:, rc, :]
                        t1 = scratch.tile([128, CD // 2], F32, tag="t1")
                        t2 = scratch.tile([128, CD // 2], F32, tag="t2")
                        nc.vector.tensor_mul(out=t1, in0=gp[:, :, 0], in1=ce)
                        nc.vector.tensor_mul(out=t2, in0=gp[:, :, 1], in1=se)
                        nc.vector.tensor_sub(out=rp[:, :, 0], in0=t1, in1=t2)
                        t3 = scratch.tile([128, CD // 2], F32, tag="t3")
                        t4 = scratch.tile([128, CD // 2], F32, tag="t4")
                        nc.vector.tensor_mul(out=t3, in0=gp[:, :, 0], in1=se)
                        nc.vector.tensor_mul(out=t4, in0=gp[:, :, 1], in1=ce)
                        nc.vector.tensor_add(out=rp[:, :, 1], in0=t3, in1=t4)

                # transpose new q/k to [hd, r] per head
                for src_t, base in ((qs, 0), (ks, HL)):
                    for h in range(HL):
                        for rc in range(RC):
                            pt = ptp.tile([128, 128], F32)
                            nc.tensor.transpose(
                                out=pt, in_=src_t[:, rc, ts(h, 128)], identity=ident[:]
                            )
                            nc.vector.tensor_copy(
                                out=q_kT[:, base + h, ts(rc, 128)], in_=pt
                            )

            # ---- attention ----
            with (
                tc.tile_pool(name="wo", bufs=1) as pwo,
                tc.tile_pool(name="kc", bufs=2) as pk,
                tc.tile_pool(name="vcp", bufs=2) as pvv,
                tc.tile_pool(name="scoreps", bufs=3, space="PSUM") as psc,
                tc.tile_pool(name="outps", bufs=2, space="PSUM") as pout,
                tc.tile_pool(name="denps", bufs=1, space="PSUM") as pden,
                tc.tile_pool(name="ptiles", bufs=3) as ppb,
                tc.tile_pool(name="small", bufs=2) as psm,
                tc.tile_pool(name="outproj", bufs=2, space="PSUM") as pop,
                tc.tile_pool(name="outsb", bufs=2) as pos,
            ):
                wo_sb = pwo.tile([128, HL, DIM], F32R)
                nc.sync.dma_start(out=wo_sb, in_=woT[:])
                for b in range(B):
                    for h in range(HL):
                        bh = b * HL + h
                        qT_bh = q_kT[:, h, b * S_NEW : (b + 1) * S_NEW]
                        out_ps = pout.tile([128, S_NEW], F32)
                        den_ps = pden.tile([1, 2 * S_NEW], F32)
                        n_pairs = (n_sc * tpc + 2) // 2
                        pend = None
                        pidx = 0

                        def emit_pending(stop):
                            vA, vB, p_pair, start = pend
                            nc.tensor.matmul(
                                out=out_ps,
                                lhsT=vA,
                                rhs=p_pair[:, 0:S_NEW],
                                start=start,
                                stop=False,
                            )
                            nc.tensor.matmul(
                                out=out_ps,
                                lhsT=vB,
                                rhs=p_pair[:, S_NEW : 2 * S_NEW],
                                start=False,
                                stop=stop,
                            )
                            nc.tensor.matmul(
                                out=den_ps,
                                lhsT=ones_t[:],
                                rhs=p_pair[:],
                                start=start,
                                stop=stop,
                            )

                        def do_pair(kA, kB, vA, vB):
                            nonlocal pend, pidx
                            s_pair = psc.tile(
                                [128, 2 * S_NEW], F32, name="s_pair", tag="s_pair"
                            )
                            nc.tensor.matmul(
                                out=s_pair[:, 0:S_NEW],
                                lhsT=kA,
                                rhs=qT_bh,
                                start=True,
                                stop=True,
                            )
                            nc.tensor.matmul(
                                out=s_pair[:, S_NEW : 2 * S_NEW],
                                lhsT=kB,
                                rhs=qT_bh,
                                start=True,
                                stop=True,
                            )
                            if pend is not None:
                                emit_pending(False)
                            p_pair = ppb.tile(
                                [128, 2 * S_NEW], BF16, name="p_pair", tag="p_pair"
                            )
                            nc.scalar.activation(
                                out=p_pair, in_=s_pair[:], func=AF.Exp, scale=SCALE
                            )
                            pend = (vA, vB, p_pair, pidx == 0)
                            pidx += 1

                        for sc in range(n_sc):
                            kT_sb = pk.tile([128, s_chunk], BF16)
                            nc.sync.dma_start(
                                out=kT_sb, in_=kTc[b, h, :, ts(sc, s_chunk)]
                            )
                            v_sb = pvv.tile([128, tpc, 128], BF16)
                            nc.sync.dma_start(out=v_sb, in_=vc[b, h, sc])
                            for tp in range(tpc // 2):
                                do_pair(
                                    kT_sb[:, ts(2 * tp, 128)],
                                    kT_sb[:, ts(2 * tp + 1, 128)],
                                    v_sb[:, 2 * tp, :],
                                    v_sb[:, 2 * tp + 1, :],
                                )
                        # the two new s-tiles form the final pair
                        do_pair(
                            q_kT[:, HL + h, b * S_NEW : b * S_NEW + 128],
                            q_kT[:, HL + h, b * S_NEW + 128 : b * S_NEW + 256],
                            vs[:, b * 2, ts(h, 128)],
                            vs[:, b * 2 + 1, ts(h, 128)],
                        )
                        emit_pending(True)
                        assert pidx == n_pairs

                        den_sb = psm.tile([1, 2 * S_NEW], F32, tag="den_sb")
                        nc.vector.tensor_copy(out=den_sb, in_=den_ps[:])
                        den_h = psm.tile([1, S_NEW], F32, tag="den_h")
                        nc.vector.tensor_add(
                            out=den_h,
                            in0=den_sb[0:1, 0:S_NEW],
                            in1=den_sb[0:1, S_NEW : 2 * S_NEW],
                        )
                        rec = psm.tile([1, S_NEW], F32, tag="rec")
                        nc.vector.reciprocal(out=rec, in_=den_h[:])
                        rec_bc = psm.tile([128, S_NEW], F32, tag="rec_bc")
                        nc.gpsimd.partition_broadcast(rec_bc[:], rec[:])
                        nc.vector.tensor_mul(
                            out=attn_sb[:, bh, :], in0=out_ps, in1=rec_bc
                        )

                    # output projection for this b (partial over this core's heads)
                    for rh in range(2):
                        out_sb = pos.tile([128, DIM], F32)
                        for oc in range(DIM // 512):
                            o_ps = pop.tile([128, 512], F32)
                            for h in range(HL):
                                nc.tensor.matmul(
                                    out=o_ps,
                                    lhsT=attn_sb[:, b * HL + h, ts(rh, 128)],
                                    rhs=wo_sb[:, h, ts(oc, 512)],
                                    start=(h == 0),
                                    stop=(h == HL - 1),
                                )
                            nc.vector.tensor_copy(out=out_sb[:, ts(oc, 512)], in_=o_ps)
                        r0 = b * S_NEW + rh * 128
                        nc.sync.dma_start(
                            out=out_d[r0 : r0 + 128, :], in_=out_sb
                        )

    nc.compile()
    return nc


_CACHE = {}


def _get_nc(s_cached, s_chunk):
    key = (s_cached, s_chunk)
    if key not in _CACHE:
        _CACHE[key] = build(s_cached, s_chunk)
    return _CACHE[key]


def make_in_maps(x, freqs, k_cache, v_cache, Wq, bq, Wk, bk, Wv, bv, Wo, bo, gq, gk,
                 s_chunk=4096):
    s_cached = k_cache.shape[1]
    n_sc = s_cached // s_chunk
    tpc = s_chunk // 128
    x2 = np.ascontiguousarray(x, dtype=np.float32).reshape(R, DIM)
    # [128, NI, R] with element (p, n, r) = xT[n*128+p, r] = x2[r, n*128+p]
    xT = np.ascontiguousarray(x2.T.reshape(NI, 128, R).transpose(1, 0, 2))
    cos = np.cos(np.asarray(freqs, dtype=np.float32))
    sin = np.sin(np.asarray(freqs, dtype=np.float32))

    def prearrange_rot(t):
        # [S_new, 64] -> [R, 192] (b-tile, head-tile) -> [128, RC, 192]
        full = np.tile(np.tile(t, (B, 1)), (1, HL))
        return np.ascontiguousarray(full.reshape(RC, 128, CD // 2).transpose(1, 0, 2))

    cosb = prearrange_rot(cos)
    sinb = prearrange_rot(sin)
    ones = np.ones((128, 1), dtype=ml_dtypes.bfloat16)
    Wq = np.asarray(Wq, dtype=np.float32)
    Wk = np.asarray(Wk, dtype=np.float32)
    Wv = np.asarray(Wv, dtype=np.float32)
    Wo = np.asarray(Wo, dtype=np.float32)
    k_cache = np.asarray(k_cache, dtype=np.float32)
    v_cache = np.asarray(v_cache, dtype=np.float32)

    def prew(Wslice):
        # W[c_slice, :].T = [DIM, CD] -> [128, NI, CD]
        return np.ascontiguousarray(
            Wslice.T.reshape(NI, 128, CD).transpose(1, 0, 2)
        )

    in_maps = []
    for c in range(NCORES):
        cs, ce = c * CD, (c + 1) * CD
        kTc = np.ascontiguousarray(
            k_cache[:, :, cs:ce]
            .reshape(B, s_cached, HL, HD)
            .transpose(0, 2, 3, 1)
            .astype(ml_dtypes.bfloat16)
        )
        # [B, HL, n_sc, 128, tpc, 128]: (b,h,sc,p,t,d) = v[b, sc*s_chunk+t*128+p, cs+h*128+d]
        vc = np.ascontiguousarray(
            v_cache[:, :, cs:ce]
            .reshape(B, n_sc, tpc, 128, HL, 128)
            .transpose(0, 4, 1, 3, 2, 5)
            .astype(ml_dtypes.bfloat16)
        )
        woT = np.ascontiguousarray(
            Wo[:, cs:ce].T.reshape(HL, 128, DIM).transpose(1, 0, 2)
        )
        in_maps.append(
            {
                "xT": xT,
                "wqT": prew(Wq[cs:ce, :]),
                "wkT": prew(Wk[cs:ce, :]),
                "wvT": prew(Wv[cs:ce, :]),
                "woT": woT,
                "kTc": kTc,
                "vc": vc,
                "cosb": cosb,
                "sinb": sinb,
                "gq": np.ascontiguousarray(gq[cs:ce])[None, :].astype(np.float32),
                "gk": np.ascontiguousarray(gk[cs:ce])[None, :].astype(np.float32),
                "bq": np.ascontiguousarray(bq[cs:ce])[None, :].astype(np.float32),
                "bk": np.ascontiguousarray(bk[cs:ce])[None, :].astype(np.float32),
                "bv": np.ascontiguousarray(bv[cs:ce])[None, :].astype(np.float32),
                "ones_in": ones,
            }
        )
    return in_maps


def kernel(x, freqs, k_cache, v_cache, Wq, bq, Wk, bk, Wv, bv, Wo, bo, gq, gk):
    s_cached = k_cache.shape[1]
    s_chunk = 4096 if s_cached % 4096 == 0 else 512
    nc = _get_nc(s_cached, s_chunk)
    in_maps = make_in_maps(
        x, freqs, k_cache, v_cache, Wq, bq, Wk, bk, Wv, bv, Wo, bo, gq, gk,
        s_chunk=s_chunk,
    )
    res = run_bass_kernel_spmd(nc, in_maps, list(range(NCORES)))
    acc = np.zeros((R, DIM), dtype=np.float64)
    for c in range(NCORES):
        acc += res.results[c]["out"].astype(np.float64)
    out = (acc + np.asarray(bo, dtype=np.float64)[None, :]).astype(np.float32)
    return out.reshape(B, S_NEW, DIM)



# revision 28
# speedup vs baseline: 1.4891x; 1.4891x over previous
"""Cached self-attention (QK-RMSNorm + RoPE + extend-cache MHA + out-proj),
tensor-parallel over heads across 8 trn2 NeuronCores.

Sharding: Wq/Wk/Wv column-sharded (3 heads = 384 dims per core), Wo
row-sharded; each core owns its slice of the KV cache. The QK RMSNorm is over
the full 3072-dim vector, so per-core partial sum-of-squares are AllReduced
(tiny [128,8] tensor). The output projection produces per-core partial sums
over the full model dim which the host reduces (the "all-reduce after the
output projection" done host-side, where it is free).

Precision: everything that streams from HBM (x, all weights, K/V cache,
rope cos/sin) is bf16; accumulation is fp32 in PSUM. Measured end-to-end
relative error ~4e-3 of absmax vs the fp32 jax reference.

Engine budget per core (cost model): PE ~148us busy (projections 46,
scores+V 85, out-proj 15); ACT ~110us (one 1024-wide exp per 4-key-tile
"quad" amortizes the fixed cost; rstd via exp(-0.5*ln(x)) keeps the whole
kernel in one activation table); DVE ~95us (drains, rope, softmax-denominator
accumulation in bf16 2x mode); Pool does the partition-sum of the
denominator + out-proj PSUM drains; DMA ~125us total (well under PE).

The softmax denominator is NOT computed on the PE (a [1,N] matmul costs the
same as a [128,N] one): exp(probs) quads are accumulated into a bf16
p_acc[128,1024] on the DVE, folded to [128,256], and partition-summed on the
Pool engine with partition_all_reduce.

Device layouts (host pre-arranges everything so every DMA is contiguous):
  xT   [128, 24, 512]    x.T partition-tiled, bf16; rows r = b*256 + s
  wqT/wkT/wvT [128, 24, 384]  W[c_slice, :].T partition-tiled, bf16
  woT  [128, 3, 3072]    Wo[:, c_slice].T partition-tiled, bf16
  kTc  [2, 3, 128, 8192] cached K head-transposed, bf16 (hd on partitions)
  vc   [2, 3, 2, 128, 32, 128] cached V pre-tiled to SBUF layout, bf16
  rope_cs [128, 2, 4, 4, 192]  RMSNorm-gain-folded rope tables, bf16:
     [:, t, 0] = g_even*cos, [:, t, 1] = g_odd*sin,
     [:, t, 2] = g_even*sin, [:, t, 3] = g_odd*cos   (t=0: q, t=1: k)
"""

import ml_dtypes
import numpy as np

import concourse.bass as bass
import concourse.mybir as mybir
import concourse.tile as tile
from concourse import bacc, bass_isa
from concourse.bass import ts
from concourse.bass_utils import run_bass_kernel_spmd
from concourse.masks import make_identity

F32 = mybir.dt.float32
BF16 = mybir.dt.bfloat16
AF = mybir.ActivationFunctionType
OP = mybir.AluOpType

B = 2
S_NEW = 256
DIM = 3072
NUM_HEADS = 24
HD = 128
EPS = 1e-6
NCORES = 8
HL = NUM_HEADS // NCORES  # heads per core: 3
CD = HL * HD  # per-core head dims: 384
R = B * S_NEW  # 512 query rows, r = b*256 + s
RC = R // 128  # 4 row chunks
NI = DIM // 128  # 24 contraction chunks
WG = 4  # weight/x DMA group size (i-chunks per DMA)
SCALE = 1.0 / np.sqrt(HD)


def build(s_cached: int, s_chunk: int, collective: bool = True):
    """Build the per-core SPMD module. s_cached/s_chunk parameterized so a
    scaled-down variant can run under CoreSim."""
    n_sc = s_cached // s_chunk
    tpc = s_chunk // 128  # 128-row s-tiles per chunk
    qpc = tpc // 4  # quads per chunk
    assert s_cached % s_chunk == 0 and s_chunk % 512 == 0
    nc = bacc.Bacc("TRN2", target_bir_lowering=False, debug=False, num_devices=NCORES)

    xT = nc.declare_dram_parameter("xT", [128, NI, R], BF16, isOutput=False)
    wqT = nc.declare_dram_parameter("wqT", [128, NI, CD], BF16, isOutput=False)
    wkT = nc.declare_dram_parameter("wkT", [128, NI, CD], BF16, isOutput=False)
    wvT = nc.declare_dram_parameter("wvT", [128, NI, CD], BF16, isOutput=False)
    woT = nc.declare_dram_parameter("woT", [128, HL, DIM], BF16, isOutput=False)
    kTc = nc.declare_dram_parameter("kTc", [B, HL, HD, s_cached], BF16, isOutput=False)
    vc = nc.declare_dram_parameter(
        "vc", [B, HL, n_sc, 128, tpc, 128], BF16, isOutput=False
    )
    rope_cs = nc.declare_dram_parameter(
        "rope_cs", [128, 2, 4, RC, CD // 2], BF16, isOutput=False
    )
    bq = nc.declare_dram_parameter("bq", [1, CD], F32, isOutput=False)
    bk = nc.declare_dram_parameter("bk", [1, CD], F32, isOutput=False)
    bv = nc.declare_dram_parameter("bv", [1, CD], F32, isOutput=False)
    out_d = nc.declare_dram_parameter("out", [R, DIM], F32, isOutput=True)

    with tile.TileContext(nc) as tc:
        with (
            tc.tile_pool(name="const", bufs=1) as const,
            tc.tile_pool(name="dram", bufs=1, space="DRAM") as dram,
            tc.tile_pool(name="qkT", bufs=1) as pqkT,
            tc.tile_pool(name="vsb", bufs=1) as pvs,
            tc.tile_pool(name="attn", bufs=1) as pattn,
            tc.tile_pool(name="wo", bufs=1) as pwo,
            tc.tile_pool(name="kc", bufs=5) as pk,
            tc.tile_pool(name="vcp", bufs=5) as pvv,
        ):
            # ---- constants (broadcasts ride the Pool SWDGE queue, off the
            # critical SP DMA queue) ----
            ident = const.tile([128, 128], BF16)
            make_identity(nc, ident)
            eps_t = const.tile([128, 1], F32)
            nc.vector.memset(eps_t, EPS)
            # Pin the activation table: the first ACT func decides the loaded
            # table; Ln forces natural_log_exp_and_others which also contains
            # Square, Exp and Copy, so the kernel never reloads tables.
            pin_t = const.tile([1, 1], F32)
            nc.scalar.activation(out=pin_t, in_=eps_t[0:1, :], func=AF.Ln)
            bcasts = {}
            for name, src in (("bq", bq), ("bk", bk), ("bv", bv)):
                t = const.tile([128, CD], F32, tag=f"bc_{name}")
                nc.gpsimd.dma_start(out=t, in_=src[:].to_broadcast((128, CD)))
                bcasts[name] = t
            crope = const.tile([128, 2, 4, RC, CD // 2], BF16)

            # persistent activations
            q_kT = pqkT.tile([128, 2 * HL, R], BF16)  # [hd, 0:3 qheads | 3:6 kheads, r]
            vs = pvs.tile([128, RC, CD], BF16)  # new V natural
            attn_sb = pattn.tile([128, B * HL, S_NEW], BF16)  # normalized attn outT

            # K/V cache chunk streamer: pools live at the outer scope so the
            # first chunks + Wo can be DMAed during the projection phase
            # (before the phase-boundary pool-close barrier).
            chunk_order = [
                (b, h, sc) for b in range(B) for h in range(HL)
                for sc in range(n_sc)
            ]
            chunk_tiles = {}

            def issue_chunk(j):
                if j >= len(chunk_order):
                    return
                cb, ch, csc = chunk_order[j]
                kt = pk.tile([128, s_chunk], BF16, tag="kt")
                nc.sync.dma_start(out=kt, in_=kTc[cb, ch, :, ts(csc, s_chunk)])
                vt = pvv.tile([128, tpc, 128], BF16, tag="vt")
                nc.sync.dma_start(out=vt, in_=vc[cb, ch, csc])
                chunk_tiles[j] = (kt, vt)

            with (
                tc.tile_pool(name="xt", bufs=1) as px,
                tc.tile_pool(name="wstream", bufs=4) as pw,
                tc.tile_pool(name="projps", bufs=6, space="PSUM") as pp,
                tc.tile_pool(name="qknat", bufs=1) as pqk,
                tc.tile_pool(name="rope", bufs=1) as prope,
                tc.tile_pool(name="scratch", bufs=2) as scratch,
                tc.tile_pool(name="stats", bufs=1) as pstats,
                tc.tile_pool(name="tps", bufs=2, space="PSUM") as ptp,
            ):
                xt = px.tile([128, NI, R], BF16)
                qs = pqk.tile([128, RC, CD], BF16, tag="qs")
                ks = pqk.tile([128, RC, CD], BF16, tag="ks")
                qr = prope.tile([128, RC, CD], BF16, tag="qr")
                kr = prope.tile([128, RC, CD], BF16, tag="kr")
                ssq = pstats.tile([128, 8], F32, tag="ssq")
                ssq_red = pstats.tile([128, 8], F32, tag="ssq_red")
                rstd = pstats.tile([128, 8], F32, tag="rstd")
                diags = pstats.tile([128, 8, 128], BF16, tag="diags")

                # Projection pass: per weight-group DMA interleaved with the
                # x-group DMA (Q first so the PE can start ~4us in), then the
                # matmul stream. ssq goes through ACT (square + accumulator)
                # to keep the DVE free for drains + rope.
                def proj_dmas(wT_d, with_x):
                    tiles = []
                    for g in range(NI // WG):
                        w_t = pw.tile([128, WG, CD], BF16, tag="w")
                        nc.sync.dma_start(out=w_t, in_=wT_d[:, ts(g, WG), :])
                        tiles.append(w_t)
                        if with_x:
                            nc.sync.dma_start(
                                out=xt[:, ts(g, WG), :], in_=xT[:, ts(g, WG), :]
                            )
                    return tiles

                def proj_alloc():
                    return [
                        pp.tile([128, CD], F32, name="projps", tag="projps")
                        for rc in range(RC)
                    ]

                def proj_mm_range(tiles, psums, i0, i1):
                    for i in range(i0, i1):
                        for rc in range(RC):
                            nc.tensor.matmul(
                                out=psums[rc],
                                lhsT=xt[:, i, ts(rc, 128)],
                                rhs=tiles[i // WG][:, i % WG, :],
                                start=(i == 0),
                                stop=(i == NI - 1),
                            )

                def proj_drains(psums, nat_out, bias_t, ssq_col):
                    for rc in range(RC):
                        nc.vector.tensor_add(
                            out=nat_out[:, rc, :], in0=psums[rc], in1=bias_t
                        )
                        if ssq_col is not None:
                            sqj = scratch.tile([128, CD], BF16, tag="sqj")
                            nc.scalar.activation(
                                out=sqj,
                                in_=nat_out[:, rc, :],
                                func=AF.Square,
                                accum_out=ssq[:, ssq_col + rc : ssq_col + rc + 1],
                            )

                def proj_mms(tiles, nat_out, bias_t, ssq_col):
                    psums = proj_alloc()
                    proj_mm_range(tiles, psums, 0, NI)
                    proj_drains(psums, nat_out, bias_t, ssq_col)

                wq_t = proj_dmas(wqT, with_x=True)
                wk_t = proj_dmas(wkT, with_x=False)
                nc.sync.dma_start(out=crope, in_=rope_cs[:])
                wv_t = proj_dmas(wvT, with_x=False)

                # RoPE is linear per channel-pair, so the per-row rstd factor
                # commutes through it: rope runs on the raw biased q/k right
                # after the drains (no AllReduce dependency), and rstd is
                # applied on the PE by the transposes (identity -> diag(rstd)).
                # Channel pairs are host-permuted to (re-half, im-half) per
                # head so every rope operand is packed bf16 (DVE 2x mode).
                def rope(ti, nat, rop):
                    for rc in range(RC):
                        gp = nat[:, rc, :].rearrange(
                            "p (h half f) -> p h half f", half=2, f=64
                        )
                        rp = rop[:, rc, :].rearrange(
                            "p (h half f) -> p h half f", half=2, f=64
                        )
                        cs3 = [
                            crope[:, ti, a, rc, :].rearrange("p (h f) -> p h f", f=64)
                            for a in range(4)
                        ]
                        ca, sb, sa, cb = cs3
                        t1 = scratch.tile([128, HL, 64], BF16, tag="t1")
                        t2 = scratch.tile([128, HL, 64], BF16, tag="t2")
                        nc.vector.tensor_mul(out=t1, in0=gp[:, :, 0, :], in1=ca)
                        nc.vector.tensor_mul(out=t2, in0=gp[:, :, 1, :], in1=sb)
                        nc.vector.tensor_sub(out=rp[:, :, 0, :], in0=t1, in1=t2)
                        t3 = scratch.tile([128, HL, 64], BF16, tag="t3")
                        t4 = scratch.tile([128, HL, 64], BF16, tag="t4")
                        nc.vector.tensor_mul(out=t3, in0=gp[:, :, 0, :], in1=sa)
                        nc.vector.tensor_mul(out=t4, in0=gp[:, :, 1, :], in1=cb)
                        nc.vector.tensor_add(out=rp[:, :, 1, :], in0=t3, in1=t4)

                proj_mms(wq_t, qs, bcasts["bq"], 0)
                rope(0, qs, qr)
                proj_mms(wk_t, ks, bcasts["bk"], 4)

                # tiny AllReduce of the norm statistics (in flight during V)
                cc_in = dram.tile([128, 8], F32)
                cc_out = dram.tile([128, 8], F32)
                nc.sync.dma_start(out=cc_in[:], in_=ssq)
                if collective:
                    nc.gpsimd.collective_compute(
                        "AllReduce",
                        OP.add,
                        replica_groups=[list(range(NCORES))],
                        ins=[cc_in.opt()],
                        outs=[cc_out.opt()],
                    )
                else:
                    nc.sync.dma_start(out=cc_out[:], in_=cc_in[:])
                nc.sync.dma_start(out=ssq_red, in_=cc_out[:])

                rope(1, ks, kr)

                # prefetch the first K/V chunks + Wo while V projects
                for j in range(4):
                    issue_chunk(j)
                wo_sb = pwo.tile([128, HL, DIM], BF16)
                nc.sync.dma_start(out=wo_sb, in_=woT[:])

                # V projection, split around the q/k transposes so the PE
                # reaches them exactly when diag(rstd) lands
                psv = proj_alloc()
                proj_mm_range(wv_t, psv, 0, NI // 2)

                # rstd = (ssq/DIM + eps)^-0.5 = exp(-0.5*ln(ssq/DIM + eps)):
                # Ln and Exp share an activation table (Sqrt does not), so the
                # whole kernel runs without a table reload.
                lns = pstats.tile([128, 8], F32, tag="lns")
                nc.scalar.activation(
                    out=lns, in_=ssq_red, func=AF.Ln, bias=eps_t, scale=1.0 / DIM
                )
                nc.scalar.activation(out=rstd, in_=lns, func=AF.Exp, scale=-0.5)
                for col in range(8):
                    nc.vector.tensor_scalar_mul(
                        out=diags[:, col, :],
                        in0=ident,
                        scalar1=rstd[:, col : col + 1],
                    )

                # transposes apply rstd via diag; 4 row-chunks batched into
                # one PSUM tile -> one DVE copy per head
                # (a plain matmul, not the PE transpose datapath: the HW
                # transpose ignores the identity operand's VALUES, a matmul
                # computes rope.T @ diag(rstd) exactly)
                for src_t, base, col0 in ((qr, 0, 0), (kr, HL, 4)):
                    for h in range(HL):
                        pt = ptp.tile([128, 512], F32)
                        for rc in range(RC):
                            nc.tensor.matmul(
                                out=pt[:, ts(rc, 128)],
                                lhsT=src_t[:, rc, ts(h, 128)],
                                rhs=diags[:, col0 + rc, :],
                                start=True,
                                stop=True,
                            )
                        nc.vector.tensor_copy(out=q_kT[:, base + h, :], in_=pt)

                proj_mm_range(wv_t, psv, NI // 2, NI)
                proj_drains(psv, vs, bcasts["bv"], None)

            # ---- attention ----
            with (
                tc.tile_pool(name="scoreps", bufs=3, space="PSUM") as psc,
                tc.tile_pool(name="outps", bufs=2, space="PSUM") as pout,
                tc.tile_pool(name="ptiles", bufs=3) as ppb,
                tc.tile_pool(name="pacc", bufs=2) as pac,
                tc.tile_pool(name="small", bufs=2) as psm,
                tc.tile_pool(name="outsb", bufs=4) as pos,
            ):
                # Output-projection emission, chopped into per-matmul closures
                # so b=0's out-proj interleaves into b=1's quad stream (one mm
                # per item rides the PE slack of the ACT-bound exp pace
                # instead of stalling the exp stream for a 8us block).
                # PSUM comes from the score pool: an out-proj group occupies
                # one of its 3 slots for ~3 items.
                def outproj_closures(b, tail):
                    cls = []
                    for rh in range(2):
                        r0 = b * S_NEW + rh * 128
                        for oc in range(DIM // 512):
                            box = {}

                            def cmm(hh, box=box, b=b, rh=rh, oc=oc, r0=r0):
                                if hh == 0:
                                    # full squad-tagged tile (same pool slot
                                    # set as the score quads); only the first
                                    # 512 columns are used
                                    box["sq"] = psc.tile(
                                        [128, 4 * S_NEW], F32,
                                        name="squad", tag="squad",
                                    )
                                    box["ps"] = box["sq"][:, 0:512]
                                nc.tensor.matmul(
                                    out=box["ps"],
                                    lhsT=attn_sb[:, b * HL + hh, ts(rh, 128)],
                                    rhs=wo_sb[:, hh, ts(oc, 512)],
                                    start=(hh == 0),
                                    stop=(hh == HL - 1),
                                )
                                if hh == HL - 1:
                                    # GPSIMD cannot read PSUM on HW: drains go
                                    # to DVE (has slack) while interleaved with
                                    # attention, to the then-idle ACT on the
                                    # tail (alternating with DVE)
                                    ob = pos.tile([128, 512], F32, tag="ob")
                                    if tail and (rh * 6 + oc) % 2 == 0:
                                        nc.scalar.copy(out=ob, in_=box["ps"])
                                    else:
                                        nc.vector.tensor_copy(
                                            out=ob, in_=box["ps"]
                                        )
                                    nc.sync.dma_start(
                                        out=out_d[r0 : r0 + 128, ts(oc, 512)],
                                        in_=ob,
                                    )

                            for hh in range(HL):
                                cls.append(lambda hh=hh, cmm=cmm: cmm(hh))
                    return cls

                op_work = []
                cj = 0  # global chunk index
                for b in range(B):
                    for h in range(HL):
                        bh = b * HL + h
                        qT_bh = q_kT[:, h, b * S_NEW : (b + 1) * S_NEW]
                        out_ps = pout.tile([128, S_NEW], F32)
                        p_acc = pac.tile([128, 4 * S_NEW], BF16)
                        pend = None
                        tidx = 0
                        n_items = n_sc * qpc + 1

                        def emit_pending(stop):
                            vls, p_t, start = pend
                            for j, v_ap in enumerate(vls):
                                nc.tensor.matmul(
                                    out=out_ps,
                                    lhsT=v_ap,
                                    rhs=p_t[:, ts(j, S_NEW)],
                                    start=start and j == 0,
                                    stop=stop and j == len(vls) - 1,
                                )

                        def do_item(kaps, vaps):
                            nonlocal pend, tidx
                            w = len(kaps) * S_NEW
                            squad = psc.tile(
                                [128, 4 * S_NEW], F32, name="squad", tag="squad"
                            )
                            for j, k_ap in enumerate(kaps):
                                nc.tensor.matmul(
                                    out=squad[:, ts(j, S_NEW)],
                                    lhsT=k_ap,
                                    rhs=qT_bh,
                                    start=True,
                                    stop=True,
                                )
                            if pend is not None:
                                emit_pending(False)
                            p_t = ppb.tile(
                                [128, 4 * S_NEW], BF16, name="p_t", tag="p_t"
                            )
                            nc.scalar.activation(
                                out=p_t[:, 0:w], in_=squad[:, 0:w],
                                func=AF.Exp, scale=SCALE,
                            )
                            if tidx == 0:
                                nc.vector.tensor_copy(
                                    out=p_acc[:, 0:w], in_=p_t[:, 0:w]
                                )
                            else:
                                nc.vector.tensor_add(
                                    out=p_acc[:, 0:w], in0=p_acc[:, 0:w],
                                    in1=p_t[:, 0:w],
                                )
                            pend = (vaps, p_t, tidx == 0)
                            tidx += 1
                            if op_work:
                                op_work.pop(0)()

                        for sc in range(n_sc):
                            kt, vt = chunk_tiles.pop(cj)
                            issue_chunk(cj + 4)
                            cj += 1
                            for qd in range(qpc):
                                do_item(
                                    [kt[:, ts(4 * qd + j, 128)] for j in range(4)],
                                    [vt[:, 4 * qd + j, :] for j in range(4)],
                                )
                        # the two new s-tiles form the final (half-width) item
                        do_item(
                            [
                                q_kT[:, HL + h, b * S_NEW : b * S_NEW + 128],
                                q_kT[:, HL + h, b * S_NEW + 128 : b * S_NEW + 256],
                            ],
                            [vs[:, b * 2, ts(h, 128)], vs[:, b * 2 + 1, ts(h, 128)]],
                        )
                        emit_pending(True)
                        assert tidx == n_items

                        # softmax denominator: fold p_acc to [128,256] on DVE
                        # (bf16 2x mode), partition-sum on Pool, reciprocal +
                        # normalize on DVE
                        f1 = psm.tile([128, 2 * S_NEW], BF16, tag="f1")
                        nc.vector.tensor_add(
                            out=f1, in0=p_acc[:, 0 : 2 * S_NEW],
                            in1=p_acc[:, 2 * S_NEW : 4 * S_NEW],
                        )
                        f2 = psm.tile([128, S_NEW], BF16, tag="f2")
                        nc.vector.tensor_add(
                            out=f2, in0=f1[:, 0:S_NEW], in1=f1[:, S_NEW : 2 * S_NEW]
                        )
                        den = psm.tile([128, S_NEW], F32, tag="den")
                        nc.gpsimd.partition_all_reduce(
                            den[:], f2[:], channels=128, reduce_op=bass_isa.ReduceOp.add
                        )
                        rec = psm.tile([128, S_NEW], F32, tag="rec")
                        nc.vector.reciprocal(out=rec, in_=den[:])
                        nc.vector.tensor_mul(
                            out=attn_sb[:, bh, :], in0=out_ps, in1=rec
                        )

                    # output projection for this b (partial over this core's
                    # heads): b=0 queued for interleaving into b=1's stream,
                    # b=1 emitted as the tail block (drains alternate between
                    # the then-idle ACT and Pool engines)
                    if b == 0:
                        op_work = outproj_closures(0, tail=False)
                    else:
                        for c in op_work:
                            c()
                        for c in outproj_closures(1, tail=True):
                            c()
                        op_work = []

    nc.compile()
    return nc


_CACHE = {}


def _get_nc(s_cached, s_chunk):
    key = (s_cached, s_chunk)
    if key not in _CACHE:
        _CACHE[key] = build(s_cached, s_chunk)
    return _CACHE[key]


def make_in_maps(x, freqs, k_cache, v_cache, Wq, bq, Wk, bk, Wv, bv, Wo, bo, gq, gk,
                 s_chunk=4096):
    s_cached = k_cache.shape[1]
    n_sc = s_cached // s_chunk
    tpc = s_chunk // 128
    x2 = np.ascontiguousarray(x, dtype=np.float32).reshape(R, DIM)
    # [128, NI, R] with element (p, n, r) = xT[n*128+p, r] = x2[r, n*128+p]
    xT = np.ascontiguousarray(
        x2.T.reshape(NI, 128, R).transpose(1, 0, 2)
    ).astype(ml_dtypes.bfloat16)
    freqs = np.asarray(freqs, dtype=np.float32)
    cos = np.cos(freqs)
    sin = np.sin(freqs)
    # Per-head channel permutation (even/"re" half first, odd/"im" half
    # second) applied consistently to Wq/Wk columns, the cached K head dim,
    # and the rope tables, so rope operands are contiguous on the device.
    # q.k dot products are invariant to a shared channel permutation.
    hperm = np.concatenate(
        [h * HD + np.concatenate([np.arange(0, HD, 2), np.arange(1, HD, 2)])
         for h in range(HL)]
    )

    def prearrange_rot(t):
        # [S_new, 64] -> [R, 192] (b-tile, head-tile) -> [128, RC, 192]
        full = np.tile(np.tile(t, (B, 1)), (1, HL))
        return np.ascontiguousarray(full.reshape(RC, 128, CD // 2).transpose(1, 0, 2))

    cosb = prearrange_rot(cos)  # [128, RC, 192]
    sinb = prearrange_rot(sin)
    Wq = np.asarray(Wq, dtype=np.float32)
    Wk = np.asarray(Wk, dtype=np.float32)
    Wv = np.asarray(Wv, dtype=np.float32)
    Wo = np.asarray(Wo, dtype=np.float32)
    gq = np.asarray(gq, dtype=np.float32)
    gk = np.asarray(gk, dtype=np.float32)
    k_cache = np.asarray(k_cache, dtype=np.float32)
    v_cache = np.asarray(v_cache, dtype=np.float32)

    def prew(Wslice, perm=None):
        # W[c_slice, :].T = [DIM, CD] -> [128, NI, CD]
        if perm is not None:
            Wslice = Wslice[perm, :]
        return np.ascontiguousarray(
            Wslice.T.reshape(NI, 128, CD).transpose(1, 0, 2)
        ).astype(ml_dtypes.bfloat16)

    in_maps = []
    for c in range(NCORES):
        cs, ce = c * CD, (c + 1) * CD
        kTc = np.ascontiguousarray(
            k_cache[:, :, cs:ce][:, :, hperm]
            .reshape(B, s_cached, HL, HD)
            .transpose(0, 2, 3, 1)
            .astype(ml_dtypes.bfloat16)
        )
        # [B, HL, n_sc, 128, tpc, 128]: (b,h,sc,p,t,d) = v[b, sc*s_chunk+t*128+p, cs+h*128+d]
        vcc = np.ascontiguousarray(
            v_cache[:, :, cs:ce]
            .reshape(B, n_sc, tpc, 128, HL, 128)
            .transpose(0, 4, 1, 3, 2, 5)
            .astype(ml_dtypes.bfloat16)
        )
        woT = np.ascontiguousarray(
            Wo[:, cs:ce].T.reshape(HL, 128, DIM).transpose(1, 0, 2)
        ).astype(ml_dtypes.bfloat16)
        # rmsnorm-gain-folded rope tables [128, 2, 4, RC, 192]
        rcs = np.empty((128, 2, 4, RC, CD // 2), dtype=np.float32)
        for ti, g in enumerate((gq[cs:ce], gk[cs:ce])):
            ge = g[0::2][None, None, :]  # [1,1,192] even-channel gains
            go = g[1::2][None, None, :]
            rcs[:, ti, 0] = ge * cosb
            rcs[:, ti, 1] = go * sinb
            rcs[:, ti, 2] = ge * sinb
            rcs[:, ti, 3] = go * cosb
        in_maps.append(
            {
                "xT": xT,
                "wqT": prew(Wq[cs:ce, :], hperm),
                "wkT": prew(Wk[cs:ce, :], hperm),
                "wvT": prew(Wv[cs:ce, :]),
                "woT": woT,
                "kTc": kTc,
                "vc": vcc,
                "rope_cs": np.ascontiguousarray(rcs).astype(ml_dtypes.bfloat16),
                "bq": np.ascontiguousarray(bq[cs:ce])[None, :].astype(np.float32),
                "bk": np.ascontiguousarray(bk[cs:ce])[None, :].astype(np.float32),
                "bv": np.ascontiguousarray(bv[cs:ce])[None, :].astype(np.float32),
            }
        )
    return in_maps


def kernel(x, freqs, k_cache, v_cache, Wq, bq, Wk, bk, Wv, bv, Wo, bo, gq, gk):
    s_cached = k_cache.shape[1]
    s_chunk = 4096 if s_cached % 4096 == 0 else 512
    nc = _get_nc(s_cached, s_chunk)
    in_maps = make_in_maps(
        x, freqs, k_cache, v_cache, Wq, bq, Wk, bk, Wv, bv, Wo, bo, gq, gk,
        s_chunk=s_chunk,
    )
    res = run_bass_kernel_spmd(nc, in_maps, list(range(NCORES)))
    acc = np.zeros((R, DIM), dtype=np.float64)
    for c in range(NCORES):
        acc += res.results[c]["out"].astype(np.float64)
    out = (acc + np.asarray(bo, dtype=np.float64)[None, :]).astype(np.float32)
    return out.reshape(B, S_NEW, DIM)
